# revision 3
# baseline (speedup 1.0000x reference)
"""Trainium2 Bass kernel for the ActorNetwork GNN problem (self-contained).

Strategy
--------
The batched graph is identical for every batch element (the reference's
"offset trick"), so the normalized adjacency P = D^-1/2 (A+I) D^-1/2
[5000 x 5000] is shared across all 16 batch elements and both GCN layers.
Per-edge gather/scatter is hostile to Trainium (descriptor-rate bound), so
the aggregation is done as a *dense* matmul with P sharded by destination
node across the 8 cores: each core holds a [5120 x 640] bf16 slice of P^T
(SBUF-resident, built on the host from edge_index) and aggregates for all
16 batch elements at once (256-wide). The hidden features H [5120, 256]
(tiny) are exchanged with an AllGather between layers.

Everything is node-sharded: core c owns true nodes [c*625, (c+1)*625),
padded to 640 (= 5 x 128). Global padded node id = c*640 + local.
"""

import numpy as np
import ml_dtypes

BF16NP = ml_dtypes.bfloat16

B, N, F, E, C, FC = 16, 5000, 512, 160000, 64, 128
NCORES = 8
NLOC = N // NCORES            # 625 true nodes per core
NPAD = 640                    # padded nodes per core (5 x 128)
NT = NPAD // 128              # node tiles per core
NG = NCORES * NPAD            # 5120 padded global nodes
KT = NG // 128                # 40 src k-tiles
HF = 16                       # hidden feature width
GB = 8                        # batch elements per partition group
NGRP = B // GB                # 2 groups
BFW = B * HF                  # 256 = (batch, feat) width
FKT = F // 128                # 4 k-tiles for the input features

_GRAPH_CACHE = {}


# --------------------------------------------------------------------------
# Host-side preprocessing (index/layout work only)
# --------------------------------------------------------------------------

def _preprocess(inputs):
    nf = np.asarray(inputs["node_features"], dtype=np.float32)   # [B, N, F]
    cf = np.asarray(inputs["col_features"], dtype=np.float32)    # [B, C, FC]
    ei = np.asarray(inputs["edge_index"])                        # [2, E] int64

    src = ei[0].astype(np.int64)
    dst = ei[1].astype(np.int64)

    # Degrees / normalization exactly as the reference (in-degree + self loop)
    deg = np.bincount(dst, minlength=N).astype(np.float64) + 1.0
    dinv = 1.0 / np.sqrt(deg)
    norm = (dinv[src] * dinv[dst]).astype(np.float32)

    # Dense P^T [src_padded_global, dst_padded_global], f32 accumulate
    pg = lambda n: (n // NLOC) * NPAD + (n % NLOC)
    PT = np.zeros((NG, NG), dtype=np.float32)
    np.add.at(PT, (pg(src), pg(dst)), norm)
    loop = np.arange(N, dtype=np.int64)
    pl = pg(loop)
    PT[pl, pl] += (dinv * dinv).astype(np.float32)

    pt_cores = [
        np.ascontiguousarray(PT[:, c * NPAD:(c + 1) * NPAD]).astype(BF16NP)
        for c in range(NCORES)
    ]

    # X^T slices: [B, F, NPAD] bf16 per core
    xt_cores = []
    for c in range(NCORES):
        xt = np.zeros((B, F, NPAD), dtype=BF16NP)
        xt[:, :, :NLOC] = nf[:, c * NLOC:(c + 1) * NLOC, :].transpose(0, 2, 1)
        xt_cores.append(xt)

    # Column features transposed: [FC, B*C] bf16 (replicated)
    cft = np.ascontiguousarray(
        cf.transpose(2, 0, 1).reshape(FC, B * C)).astype(BF16NP)

    W1 = np.asarray(inputs["W1"], np.float32)
    W2 = np.asarray(inputs["W2"], np.float32)
    fc_w = np.asarray(inputs["fc_w"], np.float32)
    fc_b = np.asarray(inputs["fc_b"], np.float32)
    cw1 = np.asarray(inputs["cw1"], np.float32)
    cb1 = np.asarray(inputs["cb1"], np.float32)
    cw2 = np.asarray(inputs["cw2"], np.float32)
    cb2 = np.asarray(inputs["cb2"], np.float32)
    b1 = np.asarray(inputs["b1"], np.float32)
    b2 = np.asarray(inputs["b2"], np.float32)

    shared = {
        "cft": cft,
        "w1": W1.astype(BF16NP),
        "wblk": np.kron(np.eye(GB, dtype=np.float32), W2).astype(BF16NP),
        "fcrep": np.kron(np.eye(GB, dtype=np.float32), fc_w).astype(BF16NP),
        "cw1": cw1.astype(BF16NP),
        "cw2": cw2.astype(BF16NP),
        "b1t": np.tile(b1, GB)[:, None].astype(np.float32),
        "b2t": np.tile(b2, GB)[:, None].astype(np.float32),
        "cb1": cb1[:, None].astype(np.float32),
        "clb": np.array([[fc_b[0] + cb2[0]]], dtype=np.float32),
    }
    return xt_cores, pt_cores, shared


# --------------------------------------------------------------------------
# Device graph (identical on all 8 cores)
# --------------------------------------------------------------------------

def _build_graph():
    from concourse import bacc
    import concourse.mybir as mybir
    import concourse.tile as tile
    from concourse.bass import ts

    f32 = mybir.dt.float32
    bf16 = mybir.dt.bfloat16
    AF = mybir.ActivationFunctionType

    nc = bacc.Bacc("TRN2", target_bir_lowering=False, debug=False,
                   num_devices=NCORES)

    xt_e = nc.dram_tensor("xt", [B, F, NPAD], bf16, kind="ExternalInput")
    pt_e = nc.dram_tensor("pt", [NG, NPAD], bf16, kind="ExternalInput")
    cft_e = nc.dram_tensor("cft", [FC, B * C], bf16, kind="ExternalInput")
    w1_e = nc.dram_tensor("w1", [F, HF], bf16, kind="ExternalInput")
    wblk_e = nc.dram_tensor("wblk", [128, 128], bf16, kind="ExternalInput")
    fcrep_e = nc.dram_tensor("fcrep", [128, GB], bf16, kind="ExternalInput")
    cw1_e = nc.dram_tensor("cw1", [FC, HF], bf16, kind="ExternalInput")
    cw2_e = nc.dram_tensor("cw2", [HF, 1], bf16, kind="ExternalInput")
    b1_e = nc.dram_tensor("b1t", [128, 1], f32, kind="ExternalInput")
    b2_e = nc.dram_tensor("b2t", [128, 1], f32, kind="ExternalInput")
    cb1_e = nc.dram_tensor("cb1", [HF, 1], f32, kind="ExternalInput")
    clb_e = nc.dram_tensor("clb", [1, 1], f32, kind="ExternalInput")
    out_e = nc.dram_tensor("out", [B, NPAD, C], f32, kind="ExternalOutput")

    rg = [list(range(NCORES))]

    with tile.TileContext(nc) as tc:
        with (
            tc.tile_pool(name="const", bufs=1) as constp,
            tc.tile_pool(name="ptp", bufs=1) as ptp,
            tc.tile_pool(name="hallp", bufs=1) as hallp,
            tc.tile_pool(name="rowsp", bufs=1) as rowsp,
            tc.tile_pool(name="xlp", bufs=1) as xlp,
            tc.tile_pool(name="xsp", bufs=8) as xsp,
            tc.tile_pool(name="stgp", bufs=3) as stgp,
            tc.tile_pool(name="dram", bufs=1, space="DRAM") as dramp,
            tc.tile_pool(name="ps", bufs=2, space="PSUM") as ps,
        ):
            # ---- constants into SBUF
            w1_sb = constp.tile([128, FKT, HF], bf16, name="w1_sb")
            nc.sync.dma_start(out=w1_sb[:],
                              in_=w1_e[:].rearrange("(k p) f -> p k f", p=128))
            wblk_sb = constp.tile([128, 128], bf16, name="wblk_sb")
            nc.sync.dma_start(out=wblk_sb[:], in_=wblk_e[:])
            fcrep_sb = constp.tile([128, GB], bf16, name="fcrep_sb")
            nc.sync.dma_start(out=fcrep_sb[:], in_=fcrep_e[:])
            cw1_sb = constp.tile([FC, HF], bf16, name="cw1_sb")
            nc.sync.dma_start(out=cw1_sb[:], in_=cw1_e[:])
            cw2_sb = constp.tile([HF, 1], bf16, name="cw2_sb")
            nc.sync.dma_start(out=cw2_sb[:], in_=cw2_e[:])
            b1_sb = constp.tile([128, 1], f32, name="b1_sb")
            nc.sync.dma_start(out=b1_sb[:], in_=b1_e[:])
            b2_sb = constp.tile([128, 1], f32, name="b2_sb")
            nc.sync.dma_start(out=b2_sb[:], in_=b2_e[:])
            cb1_sb = constp.tile([HF, 1], f32, name="cb1_sb")
            nc.sync.dma_start(out=cb1_sb[:], in_=cb1_e[:])
            clb_sb = constp.tile([1, 1], f32, name="clb_sb")
            nc.sync.dma_start(out=clb_sb[:], in_=clb_e[:])
            cft_sb = constp.tile([FC, B * C], bf16, name="cft_sb")
            nc.sync.dma_start(out=cft_sb[:], in_=cft_e[:])
            ones_sb = constp.tile([1, 128], bf16, name="ones_sb")
            nc.vector.memset(ones_sb[:], 1.0)

            # ---- P^T resident in SBUF: [128, KT, NPAD]
            pt_sb = ptp.tile([128, KT, NPAD], bf16, name="pt_sb")
            for kt in range(KT):
                nc.sync.dma_start(out=pt_sb[:, kt, :],
                                  in_=pt_e[ts(kt, 128), :])

            # ---- matmul1: H1[node, (b,f)] = X @ W1  (node-major direct)
            h1_rows = rowsp.tile([128, NT, BFW], bf16, tag="hrows",
                                 name="h1_rows")
            for b in range(B):
                x_tiles = []
                for kt in range(FKT):
                    x_t = xsp.tile([128, NPAD], bf16, tag="xt",
                                   name=f"x_{b}_{kt}")
                    nc.sync.dma_start(out=x_t[:], in_=xt_e[b, ts(kt, 128), :])
                    x_tiles.append(x_t)
                for t in range(NT):
                    mp = ps.tile([128, HF], f32, tag="small", bufs=4,
                                 name=f"mm1_{b}_{t}")
                    for kt in range(FKT):
                        nc.tensor.matmul(mp[:],
                                         lhsT=x_tiles[kt][:, ts(t, 128)],
                                         rhs=w1_sb[:, kt, :],
                                         start=(kt == 0), stop=(kt == FKT - 1))
                    nc.scalar.copy(out=h1_rows[:, t, ts(b, HF)], in_=mp[:])

            # ---- column MLP (replicated on every core; tiny)
            colp = ps.tile([HF, B * C], f32, tag="big", bufs=2, name="colp")
            for h in range(2):
                nc.tensor.matmul(colp[:, ts(h, 512)], lhsT=cw1_sb[:],
                                 rhs=cft_sb[:, ts(h, 512)],
                                 start=True, stop=True)
            hcol_sb = constp.tile([HF, B * C], bf16, name="hcol_sb")
            nc.scalar.activation(out=hcol_sb[:], in_=colp[:], func=AF.Relu,
                                 bias=cb1_sb[:, 0:1])
            clp = ps.tile([1, B * C], f32, tag="big", bufs=2, name="clp")
            for h in range(2):
                nc.tensor.matmul(clp[:, ts(h, 512)], lhsT=cw2_sb[:],
                                 rhs=hcol_sb[:, ts(h, 512)],
                                 start=True, stop=True)
            cl_sb = constp.tile([1, B * C], bf16, name="cl_sb")
            nc.scalar.activation(out=cl_sb[:], in_=clp[:], func=AF.Identity,
                                 bias=clb_sb[:, 0:1])

            # ---- two GCN layers: AllGather H -> dense aggregation
            xl_prev = None
            for layer in range(2):
                src_rows = h1_rows if layer == 0 else h2_rows  # noqa: F821
                ag_in = dramp.tile([NPAD, BFW], bf16, name=f"ag_in{layer}")
                ag_out = dramp.tile([NG, BFW], bf16, addr_space="Shared",
                                    name=f"ag_out{layer}")
                nc.gpsimd.dma_start(
                    out=ag_in[:].rearrange("(t p) f -> p t f", p=128),
                    in_=src_rows[:])
                nc.gpsimd.collective_compute(
                    "AllGather",
                    mybir.AluOpType.bypass,
                    replica_groups=rg,
                    ins=[ag_in[:].opt()],
                    outs=[ag_out[:].opt()],
                )
                h_all = hallp.tile([128, KT, BFW], bf16, tag="hall",
                                   name=f"hall{layer}")
                for kt in range(KT):
                    nc.sync.dma_start(out=h_all[:, kt, :],
                                      in_=ag_out[ts(kt, 128), :])

                bias_sb = b1_sb if layer == 0 else b2_sb
                xl = []
                for g in range(NGRP):
                    ap_ = ps.tile([128, NPAD], f32, tag="big", bufs=2,
                                  name=f"agg{layer}_{g}")
                    for kt in range(KT):
                        lhs = h_all[:, kt, ts(g, 128)]
                        nc.tensor.matmul(ap_[:, 0:512], lhsT=lhs,
                                         rhs=pt_sb[:, kt, 0:512],
                                         start=(kt == 0), stop=(kt == KT - 1))
                        nc.tensor.matmul(ap_[:, 512:NPAD], lhsT=lhs,
                                         rhs=pt_sb[:, kt, 512:NPAD],
                                         start=(kt == 0), stop=(kt == KT - 1))
                    x_g = xlp.tile([128, NPAD], bf16, tag=f"xl{g}",
                                   name=f"xl{layer}_{g}")
                    nc.scalar.activation(out=x_g[:], in_=ap_[:], func=AF.Relu,
                                         bias=bias_sb[:, 0:1])
                    xl.append(x_g)

                if layer == 0:
                    # H2 = X1 @ blockdiag(W2), node-major directly
                    h2_rows = rowsp.tile([128, NT, BFW], bf16, tag="hrows2",
                                         name="h2_rows")
                    for g in range(NGRP):
                        for t in range(NT):
                            mp2 = ps.tile([128, 128], f32, tag="small", bufs=4,
                                          name=f"mm2_{g}_{t}")
                            nc.tensor.matmul(mp2[:],
                                             lhsT=xl[g][:, ts(t, 128)],
                                             rhs=wblk_sb[:],
                                             start=True, stop=True)
                            nc.scalar.copy(out=h2_rows[:, t, ts(g, 128)],
                                           in_=mp2[:])
                xl_prev = xl

            # ---- head: node logits + joint broadcast-add
            nls = []
            for g in range(NGRP):
                nlt = rowsp.tile([128, NT, GB], f32, tag=f"nl{g}",
                                 name=f"nl_{g}")
                for t in range(NT):
                    np_ = ps.tile([128, GB], f32, tag="small", bufs=4, name=f"nlp_{g}_{t}")
                    nc.tensor.matmul(np_[:], lhsT=xl_prev[g][:, ts(t, 128)],
                                     rhs=fcrep_sb[:], start=True, stop=True)
                    nc.scalar.copy(out=nlt[:, t, :], in_=np_[:])
                nls.append(nlt)

            for b in range(B):
                g, j = b // GB, b % GB
                stage = stgp.tile([128, NT, C], f32, tag="stage",
                                  name=f"stage_{b}")
                for t in range(NT):
                    jp = ps.tile([128, C], f32, tag="small", bufs=4,
                                 name=f"jp_{b}_{t}")
                    nc.tensor.matmul(jp[:], lhsT=ones_sb[:],
                                     rhs=cl_sb[0:1, ts(b, C)],
                                     start=True, stop=True)
                    if t % 2 == 0:
                        nc.scalar.activation(out=stage[:, t, :], in_=jp[:],
                                             func=AF.Identity,
                                             bias=nls[g][:, t, j:j + 1])
                    else:
                        nc.vector.tensor_add(out=stage[:, t, :], in0=jp[:],
                                             in1=nls[g][:, t, j:j + 1]
                                             .to_broadcast([128, C]))
                nc.sync.dma_start(
                    out=out_e[b].rearrange("(t p) c -> p t c", p=128),
                    in_=stage[:])

    nc.compile()
    return nc


def _get_graph():
    if "nc" not in _GRAPH_CACHE:
        _GRAPH_CACHE["nc"] = _build_graph()
    return _GRAPH_CACHE["nc"]


# --------------------------------------------------------------------------
# Entry point
# --------------------------------------------------------------------------

def _run(inputs, trace=False):
    from concourse.bass_utils import run_bass_kernel_spmd

    xt_cores, pt_cores, shared = _preprocess(inputs)
    nc = _get_graph()
    in_maps = []
    for c in range(NCORES):
        m = dict(shared)
        m["xt"] = xt_cores[c]
        m["pt"] = pt_cores[c]
        in_maps.append(m)
    res = run_bass_kernel_spmd(nc, in_maps, core_ids=list(range(NCORES)),
                               trace=trace)
    out = np.zeros((B, N, C), dtype=np.float32)
    for c in range(NCORES):
        out[:, c * NLOC:(c + 1) * NLOC, :] = \
            np.asarray(res.results[c]["out"])[:, :NLOC, :]
    return out.reshape(B, N * C), res


def kernel(**inputs) -> np.ndarray:
    out, _ = _run(inputs, trace=False)
    return out


# revision 4
# speedup vs baseline: 1.2736x; 1.2736x over previous
"""Trainium2 Bass kernel for the ActorNetwork GNN problem (self-contained).

Strategy
--------
The batched graph is identical for every batch element (the reference's
"offset trick"), so the normalized adjacency P = D^-1/2 (A+I) D^-1/2
[5000 x 5000] is shared across all 16 batch elements and both GCN layers.
Per-edge gather/scatter is hostile to Trainium (descriptor-rate bound), so
the aggregation is done as a *dense* matmul with P sharded by destination
node across the 8 cores: each core holds a [5120 x 640] bf16 slice of P^T
(SBUF-resident, built on the host from edge_index) and aggregates for all
16 batch elements at once (256-wide). The hidden features H [5120, 256]
(tiny) are exchanged with an AllGather between layers.

Everything is node-sharded: core c owns true nodes [c*625, (c+1)*625),
padded to 640 (= 5 x 128). Global padded node id = c*640 + local.
"""

import numpy as np
import ml_dtypes

BF16NP = ml_dtypes.bfloat16

B, N, F, E, C, FC = 16, 5000, 512, 160000, 64, 128
NCORES = 8
NLOC = N // NCORES            # 625 true nodes per core
NPAD = 640                    # padded nodes per core (5 x 128)
NT = NPAD // 128              # node tiles per core
NG = NCORES * NPAD            # 5120 padded global nodes
KT = NG // 128                # 40 src k-tiles
HF = 16                       # hidden feature width
GB = 8                        # batch elements per partition group
NGRP = B // GB                # 2 groups
BFW = B * HF                  # 256 = (batch, feat) width
FKT = F // 128                # 4 k-tiles for the input features

_GRAPH_CACHE = {}


# --------------------------------------------------------------------------
# Host-side preprocessing (index/layout work only)
# --------------------------------------------------------------------------

def _preprocess(inputs):
    nf = np.asarray(inputs["node_features"], dtype=np.float32)   # [B, N, F]
    cf = np.asarray(inputs["col_features"], dtype=np.float32)    # [B, C, FC]
    ei = np.asarray(inputs["edge_index"])                        # [2, E] int64

    src = ei[0].astype(np.int64)
    dst = ei[1].astype(np.int64)

    # Degrees / normalization exactly as the reference (in-degree + self loop)
    deg = np.bincount(dst, minlength=N).astype(np.float64) + 1.0
    dinv = 1.0 / np.sqrt(deg)
    norm = (dinv[src] * dinv[dst]).astype(np.float32)

    # Dense P^T [src_padded_global, dst_padded_global], f32 accumulate
    pg = lambda n: (n // NLOC) * NPAD + (n % NLOC)
    PT = np.zeros((NG, NG), dtype=np.float32)
    np.add.at(PT, (pg(src), pg(dst)), norm)
    loop = np.arange(N, dtype=np.int64)
    pl = pg(loop)
    PT[pl, pl] += (dinv * dinv).astype(np.float32)

    pt_cores = [
        np.ascontiguousarray(PT[:, c * NPAD:(c + 1) * NPAD]).astype(BF16NP)
        for c in range(NCORES)
    ]

    # X^T slices: [B, F, NPAD] bf16 per core
    xt_cores = []
    for c in range(NCORES):
        xt = np.zeros((B, F, NPAD), dtype=BF16NP)
        xt[:, :, :NLOC] = nf[:, c * NLOC:(c + 1) * NLOC, :].transpose(0, 2, 1)
        xt_cores.append(xt)

    # Column features transposed: [FC, B*C] bf16 (replicated)
    cft = np.ascontiguousarray(
        cf.transpose(2, 0, 1).reshape(FC, B * C)).astype(BF16NP)

    W1 = np.asarray(inputs["W1"], np.float32)
    W2 = np.asarray(inputs["W2"], np.float32)
    fc_w = np.asarray(inputs["fc_w"], np.float32)
    fc_b = np.asarray(inputs["fc_b"], np.float32)
    cw1 = np.asarray(inputs["cw1"], np.float32)
    cb1 = np.asarray(inputs["cb1"], np.float32)
    cw2 = np.asarray(inputs["cw2"], np.float32)
    cb2 = np.asarray(inputs["cb2"], np.float32)
    b1 = np.asarray(inputs["b1"], np.float32)
    b2 = np.asarray(inputs["b2"], np.float32)

    shared = {
        "cft": cft,
        "w1": W1.astype(BF16NP),
        "wblk": np.kron(np.eye(GB, dtype=np.float32), W2).astype(BF16NP),
        "fcrep": np.kron(np.eye(GB, dtype=np.float32), fc_w).astype(BF16NP),
        "cw1": cw1.astype(BF16NP),
        "cw2": cw2.astype(BF16NP),
        "b1t": np.tile(b1, GB)[:, None].astype(np.float32),
        "b2t": np.tile(b2, GB)[:, None].astype(np.float32),
        "cb1": cb1[:, None].astype(np.float32),
        "clb": np.array([[fc_b[0] + cb2[0]]], dtype=np.float32),
    }
    return xt_cores, pt_cores, shared


# --------------------------------------------------------------------------
# Device graph (identical on all 8 cores)
# --------------------------------------------------------------------------

def _build_graph():
    from concourse import bacc
    import concourse.mybir as mybir
    import concourse.tile as tile
    from concourse.bass import ts

    f32 = mybir.dt.float32
    bf16 = mybir.dt.bfloat16
    AF = mybir.ActivationFunctionType

    nc = bacc.Bacc("TRN2", target_bir_lowering=False, debug=False,
                   num_devices=NCORES)

    xt_e = nc.dram_tensor("xt", [B, F, NPAD], bf16, kind="ExternalInput")
    pt_e = nc.dram_tensor("pt", [NG, NPAD], bf16, kind="ExternalInput")
    cft_e = nc.dram_tensor("cft", [FC, B * C], bf16, kind="ExternalInput")
    w1_e = nc.dram_tensor("w1", [F, HF], bf16, kind="ExternalInput")
    wblk_e = nc.dram_tensor("wblk", [128, 128], bf16, kind="ExternalInput")
    fcrep_e = nc.dram_tensor("fcrep", [128, GB], bf16, kind="ExternalInput")
    cw1_e = nc.dram_tensor("cw1", [FC, HF], bf16, kind="ExternalInput")
    cw2_e = nc.dram_tensor("cw2", [HF, 1], bf16, kind="ExternalInput")
    b1_e = nc.dram_tensor("b1t", [128, 1], f32, kind="ExternalInput")
    b2_e = nc.dram_tensor("b2t", [128, 1], f32, kind="ExternalInput")
    cb1_e = nc.dram_tensor("cb1", [HF, 1], f32, kind="ExternalInput")
    clb_e = nc.dram_tensor("clb", [1, 1], f32, kind="ExternalInput")
    out_e = nc.dram_tensor("out", [B, NPAD, C], f32, kind="ExternalOutput")

    rg = [list(range(NCORES))]

    with tile.TileContext(nc) as tc:
        with (
            tc.tile_pool(name="const", bufs=1) as constp,
            tc.tile_pool(name="ptp", bufs=1) as ptp,
            tc.tile_pool(name="hallp", bufs=1) as hallp,
            tc.tile_pool(name="rowsp", bufs=1) as rowsp,
            tc.tile_pool(name="xlp", bufs=1) as xlp,
            tc.tile_pool(name="xsp", bufs=8) as xsp,
            tc.tile_pool(name="stgp", bufs=3) as stgp,
            tc.tile_pool(name="dram", bufs=1, space="DRAM") as dramp,
            tc.tile_pool(name="ps", bufs=2, space="PSUM") as ps,
        ):
            # ---- constants into SBUF
            w1_sb = constp.tile([128, FKT, HF], bf16, name="w1_sb")
            nc.sync.dma_start(out=w1_sb[:],
                              in_=w1_e[:].rearrange("(k p) f -> p k f", p=128))
            wblk_sb = constp.tile([128, 128], bf16, name="wblk_sb")
            nc.sync.dma_start(out=wblk_sb[:], in_=wblk_e[:])
            fcrep_sb = constp.tile([128, GB], bf16, name="fcrep_sb")
            nc.sync.dma_start(out=fcrep_sb[:], in_=fcrep_e[:])
            cw1_sb = constp.tile([FC, HF], bf16, name="cw1_sb")
            nc.sync.dma_start(out=cw1_sb[:], in_=cw1_e[:])
            cw2_sb = constp.tile([HF, 1], bf16, name="cw2_sb")
            nc.sync.dma_start(out=cw2_sb[:], in_=cw2_e[:])
            b1_sb = constp.tile([128, 1], f32, name="b1_sb")
            nc.sync.dma_start(out=b1_sb[:], in_=b1_e[:])
            b2_sb = constp.tile([128, 1], f32, name="b2_sb")
            nc.sync.dma_start(out=b2_sb[:], in_=b2_e[:])
            cb1_sb = constp.tile([HF, 1], f32, name="cb1_sb")
            nc.sync.dma_start(out=cb1_sb[:], in_=cb1_e[:])
            clb_sb = constp.tile([1, 1], f32, name="clb_sb")
            nc.sync.dma_start(out=clb_sb[:], in_=clb_e[:])
            cft_sb = constp.tile([FC, B * C], bf16, name="cft_sb")
            nc.sync.dma_start(out=cft_sb[:], in_=cft_e[:])
            ones_sb = constp.tile([1, 128], bf16, name="ones_sb")
            nc.vector.memset(ones_sb[:], 1.0)

            # ---- matmul1: H1[node, (b,f)] = X @ W1  (node-major direct)
            # X loads come first so PE can start immediately; one DMA per b.
            h1_rows = rowsp.tile([128, NT, BFW], bf16, tag="hrows",
                                 name="h1_rows")
            x_tiles = []
            for b in range(B):
                x_t = xsp.tile([128, FKT, NPAD], bf16, tag="xt",
                               name=f"x_{b}")
                nc.sync.dma_start(
                    out=x_t[:],
                    in_=xt_e[b].rearrange("(k p) n -> p k n", p=128))
                x_tiles.append(x_t)

            # ---- P^T resident in SBUF: [128, KT, NPAD] (4 big DMAs)
            pt_sb = ptp.tile([128, KT, NPAD], bf16, name="pt_sb")
            for q in range(4):
                nc.sync.dma_start(
                    out=pt_sb[:, ts(q, KT // 4), :],
                    in_=pt_e[q * (NG // 4):(q + 1) * (NG // 4), :]
                    .rearrange("(t p) d -> p t d", p=128))

            for b in range(B):
                mp = ps.tile([128, NT * HF], f32, tag="mm1b", bufs=3,
                             name=f"mm1_{b}")
                for t in range(NT):
                    for kt in range(FKT):
                        nc.tensor.matmul(mp[:, ts(t, HF)],
                                         lhsT=x_tiles[b][:, kt, ts(t, 128)],
                                         rhs=w1_sb[:, kt, :],
                                         start=(kt == 0), stop=(kt == FKT - 1))
                nc.scalar.copy(out=h1_rows[:, :, ts(b, HF)],
                               in_=mp[:].rearrange("p (t f) -> p t f", t=NT))

            # ---- column MLP (replicated on every core; tiny)
            colp = ps.tile([HF, B * C], f32, tag="big", bufs=2, name="colp")
            for h in range(2):
                nc.tensor.matmul(colp[:, ts(h, 512)], lhsT=cw1_sb[:],
                                 rhs=cft_sb[:, ts(h, 512)],
                                 start=True, stop=True)
            hcol_sb = constp.tile([HF, B * C], bf16, name="hcol_sb")
            nc.scalar.activation(out=hcol_sb[:], in_=colp[:], func=AF.Relu,
                                 bias=cb1_sb[:, 0:1])
            clp = ps.tile([1, B * C], f32, tag="big", bufs=2, name="clp")
            for h in range(2):
                nc.tensor.matmul(clp[:, ts(h, 512)], lhsT=cw2_sb[:],
                                 rhs=hcol_sb[:, ts(h, 512)],
                                 start=True, stop=True)
            cl_sb = constp.tile([1, B * C], bf16, name="cl_sb")
            nc.scalar.activation(out=cl_sb[:], in_=clp[:], func=AF.Identity,
                                 bias=clb_sb[:, 0:1])

            # ---- two GCN layers: AllGather H (split by batch group, so the
            # collective overlaps compute) -> dense aggregation
            xl_prev = None
            for layer in range(2):
                src_rows = h1_rows if layer == 0 else h2_rows  # noqa: F821
                h_all_g = []
                for g in range(NGRP):
                    ag_in = dramp.tile([NPAD, GB * HF], bf16,
                                       name=f"ag_in{layer}_{g}")
                    ag_out = dramp.tile([NG, GB * HF], bf16,
                                        addr_space="Shared",
                                        name=f"ag_out{layer}_{g}")
                    nc.gpsimd.dma_start(
                        out=ag_in[:].rearrange("(t p) f -> p t f", p=128),
                        in_=src_rows[:, :, ts(g, GB * HF)])
                    nc.gpsimd.collective_compute(
                        "AllGather",
                        mybir.AluOpType.bypass,
                        replica_groups=rg,
                        ins=[ag_in[:].opt()],
                        outs=[ag_out[:].opt()],
                    )
                    h_all = hallp.tile([128, KT, GB * HF], bf16,
                                       tag=f"hall{g}", name=f"hall{layer}_{g}")
                    for q in range(2):
                        nc.sync.dma_start(
                            out=h_all[:, ts(q, KT // 2), :],
                            in_=ag_out[q * (NG // 2):(q + 1) * (NG // 2), :]
                            .rearrange("(t p) f -> p t f", p=128))
                    h_all_g.append(h_all)

                bias_sb = b1_sb if layer == 0 else b2_sb
                xl = []
                for g in range(NGRP):
                    ap_ = ps.tile([128, NPAD], f32, tag="big", bufs=2,
                                  name=f"agg{layer}_{g}")
                    for kt in range(KT):
                        lhs = h_all_g[g][:, kt, :]
                        nc.tensor.matmul(ap_[:, 0:512], lhsT=lhs,
                                         rhs=pt_sb[:, kt, 0:512],
                                         start=(kt == 0), stop=(kt == KT - 1))
                        nc.tensor.matmul(ap_[:, 512:NPAD], lhsT=lhs,
                                         rhs=pt_sb[:, kt, 512:NPAD],
                                         start=(kt == 0), stop=(kt == KT - 1))
                    x_g = xlp.tile([128, NPAD], bf16, tag=f"xl{g}",
                                   name=f"xl{layer}_{g}")
                    nc.scalar.activation(out=x_g[:], in_=ap_[:], func=AF.Relu,
                                         bias=bias_sb[:, 0:1])
                    xl.append(x_g)

                    if layer == 0:
                        # H2 = X1 @ blockdiag(W2), node-major directly
                        if g == 0:
                            h2_rows = rowsp.tile([128, NT, BFW], bf16,
                                                 tag="hrows2", name="h2_rows")
                        mp2 = ps.tile([128, NPAD], f32, tag="big", bufs=2,
                                      name=f"mm2_{g}")
                        for t in range(NT):
                            nc.tensor.matmul(mp2[:, ts(t, 128)],
                                             lhsT=x_g[:, ts(t, 128)],
                                             rhs=wblk_sb[:],
                                             start=True, stop=True)
                        nc.scalar.copy(
                            out=h2_rows[:, :, ts(g, 128)],
                            in_=mp2[:].rearrange("p (t f) -> p t f", t=NT))
                xl_prev = xl

            # ---- head: node logits + joint broadcast-add
            nls = []
            for g in range(NGRP):
                nlt = rowsp.tile([128, NT, GB], f32, tag=f"nl{g}",
                                 name=f"nl_{g}")
                np_ = ps.tile([128, NT * GB], f32, tag="mm1b", bufs=3,
                              name=f"nlp_{g}")
                for t in range(NT):
                    nc.tensor.matmul(np_[:, ts(t, GB)],
                                     lhsT=xl_prev[g][:, ts(t, 128)],
                                     rhs=fcrep_sb[:], start=True, stop=True)
                nc.scalar.copy(out=nlt[:],
                               in_=np_[:].rearrange("p (t f) -> p t f", t=NT))
                nls.append(nlt)

            for b in range(B):
                g, j = b // GB, b % GB
                stage = stgp.tile([128, NT, C], f32, tag="stage",
                                  name=f"stage_{b}")
                jp = ps.tile([128, NT * C], f32, tag="mm1b", bufs=3,
                             name=f"jp_{b}")
                for t in range(NT):
                    nc.tensor.matmul(jp[:, ts(t, C)], lhsT=ones_sb[:],
                                     rhs=cl_sb[0:1, ts(b, C)],
                                     start=True, stop=True)
                nc.vector.tensor_add(
                    out=stage[:],
                    in0=jp[:].rearrange("p (t c) -> p t c", t=NT),
                    in1=nls[g][:, :, j:j + 1].to_broadcast([128, NT, C]))
                nc.sync.dma_start(
                    out=out_e[b].rearrange("(t p) c -> p t c", p=128),
                    in_=stage[:])

    nc.compile()
    return nc


def _get_graph():
    if "nc" not in _GRAPH_CACHE:
        _GRAPH_CACHE["nc"] = _build_graph()
    return _GRAPH_CACHE["nc"]


# --------------------------------------------------------------------------
# Entry point
# --------------------------------------------------------------------------

def _run(inputs, trace=False):
    from concourse.bass_utils import run_bass_kernel_spmd

    xt_cores, pt_cores, shared = _preprocess(inputs)
    nc = _get_graph()
    in_maps = []
    for c in range(NCORES):
        m = dict(shared)
        m["xt"] = xt_cores[c]
        m["pt"] = pt_cores[c]
        in_maps.append(m)
    res = run_bass_kernel_spmd(nc, in_maps, core_ids=list(range(NCORES)),
                               trace=trace)
    out = np.zeros((B, N, C), dtype=np.float32)
    for c in range(NCORES):
        out[:, c * NLOC:(c + 1) * NLOC, :] = \
            np.asarray(res.results[c]["out"])[:, :NLOC, :]
    return out.reshape(B, N * C), res


def kernel(**inputs) -> np.ndarray:
    out, _ = _run(inputs, trace=False)
    return out
